# revision 40
# baseline (speedup 1.0000x reference)
"""Trainium2 Bass kernel for nn_ARDConv (MoE-routed dynamic conv).

Data-parallel over batch: 16 images -> 8 cores x 2 images. No collectives.

Per image (C=O=128 channels on partitions, L=H*W=4096 on free dim):
  - global-context + channel gates via pooled matmul chains
  - spatial gate: channel max/mean via PE transposes + free-dim reduces,
    7x7 conv as a 98-partition patch matmul (patches built by window DMAs)
  - router: dilated 3x3 conv as 9 shifted-tap matmuls (weights tiled 4x so
    lf comes out replicated for the 4 experts), alpha via row-tiled K=32
    matmuls, softmax-over-K folded into exp + reciprocal
  - main convs: static folded into the 4 experts host-side
    (softmax weights sum to 1), 4 experts x 9 taps bf16 matmuls per L-chunk
  - GroupNorm via accumulated sums/sumsq + tiny group matmuls; SE + residual
    folded into one per-channel affine final pass.
"""

import dataclasses
import numpy as np
import ml_dtypes

bf16 = ml_dtypes.bfloat16

B, C, H, W = 16, 128, 64, 64
O, HID, CGD, SE = 128, 32, 16, 16
K = 4
TAU = 1.5
EPS = 1e-5
GN_GROUPS = 8
L = H * W            # 4096
NCORES = 8
BL = B // NCORES     # 2 images per core
NCH = 8              # L-chunks per image
CHL = L // NCH       # 512
HP = H + 4           # padded pitch for x2 (pad=2 each side): 68
MPP = 70             # padded pitch for spatial maps (pad=3): 70

_CACHE = {}


def _build(debug=False):
    import concourse.bass as bass
    import concourse.tile as tile
    from concourse import bacc, mybir
    from contextlib import ExitStack

    F32 = mybir.dt.float32
    BF = mybir.dt.bfloat16
    AF = mybir.ActivationFunctionType
    ALU = mybir.AluOpType
    AX = mybir.AxisListType

    nc = bacc.Bacc()

    def din(name, shape, dt=F32):
        return nc.dram_tensor(name, shape, dt, kind="ExternalInput")

    x_d = din("x", [BL, C, L])
    wexp_d = din("wexp", [C, K * 9 * O], BF)       # [c, (k,tap,o)]
    wrc_d = din("wrc", [C, 9 * O], BF)             # [c, (tap, 4*HID)]
    brc_d = din("brc", [O, 1])
    wgate_d = din("wgate", [K * HID, O], BF)       # rows 32k..32k+32 = expert k lhsT
    bgate_d = din("bgate", [O, K])
    wgc1_d = din("wgc1", [C, HID])
    wgc2_d = din("wgc2", [HID, C])
    wcg1_d = din("wcg1", [C, CGD])
    bcg1_d = din("bcg1", [CGD, 1])
    wcg2_d = din("wcg2", [CGD, C])
    bcg2_d = din("bcg2", [C, 1])
    wrg_d = din("wrg", [C, 4 * HID])
    wsg_d = din("wsg", [98, 1], BF)
    wse1_d = din("wse1", [O, SE])
    bse1_d = din("bse1", [SE, 1])
    wse2_d = din("wse2", [SE, O])
    bse2_d = din("bse2", [O, 1])
    gng_d = din("gng", [O, 1])
    gnb_d = din("gnb", [O, 1])
    biaso_d = din("biaso", [O, 1])
    gmask_d = din("gmask", [O, GN_GROUPS])
    gmaskT_d = din("gmaskT", [GN_GROUPS, O])
    identb_d = din("identb", [128, 128], BF)
    onesbc_d = din("onesbc", [1, 128], BF)
    onescol_d = din("onescol", [C, 1], BF)
    iscale_d = din("iscale", [1, 1])
    zz_d = din("zz", [2, MPP * MPP], BF)

    out_d = nc.dram_tensor("out", [BL, O, L], F32, kind="ExternalOutput")
    dbg = {}
    if debug:
        dbg["outpre"] = nc.dram_tensor("d_outpre", [BL, O, L], F32, kind="ExternalOutput")
        dbg["satt"] = nc.dram_tensor("d_satt", [BL, 1, L], F32, kind="ExternalOutput")
        dbg["gc"] = nc.dram_tensor("d_gc", [BL, C, 2], F32, kind="ExternalOutput")
        dbg["gf"] = nc.dram_tensor("d_gf", [BL, 128, 1], F32, kind="ExternalOutput")
        dbg["x2s"] = nc.dram_tensor("d_x2s", [BL, C, 1], F32, kind="ExternalOutput")

    with tile.TileContext(nc) as tc, ExitStack() as ctx:
        cst = ctx.enter_context(tc.tile_pool(name="cst", bufs=1))
        big = ctx.enter_context(tc.tile_pool(name="big", bufs=1))
        wrk = ctx.enter_context(tc.tile_pool(name="wrk", bufs=1))
        sm = ctx.enter_context(tc.tile_pool(name="sm", bufs=1))
        pbig = ctx.enter_context(tc.tile_pool(name="pbig", bufs=5, space="PSUM"))
        pbc = ctx.enter_context(tc.tile_pool(name="pbc", bufs=1, space="PSUM"))
        paux = ctx.enter_context(tc.tile_pool(name="paux", bufs=2, space="PSUM"))

        def ld(dram, shape, dt=F32, pool=cst, tag=None):
            t = pool.tile(shape, dt, tag=tag or dram.name)
            nc.gpsimd.dma_start(t[:], dram[:])
            return t

        wexp = ld(wexp_d, [C, K * 9 * O], BF)
        wrc = ld(wrc_d, [C, 9 * O], BF)
        brc = ld(brc_d, [O, 1])
        wgate = ld(wgate_d, [K * HID, O], BF)
        bgate = ld(bgate_d, [O, K])
        wgc1 = ld(wgc1_d, [C, HID])
        wgc2 = ld(wgc2_d, [HID, C])
        wcg1 = ld(wcg1_d, [C, CGD])
        bcg1 = ld(bcg1_d, [CGD, 1])
        wcg2 = ld(wcg2_d, [CGD, C])
        bcg2 = ld(bcg2_d, [C, 1])
        wrg = ld(wrg_d, [C, 4 * HID])
        wsg = ld(wsg_d, [98, 1], BF)
        wse1 = ld(wse1_d, [O, SE])
        bse1 = ld(bse1_d, [SE, 1])
        wse2 = ld(wse2_d, [SE, O])
        bse2 = ld(bse2_d, [O, 1])
        gng = ld(gng_d, [O, 1])
        gnb = ld(gnb_d, [O, 1])
        biaso = ld(biaso_d, [O, 1])
        gmask = ld(gmask_d, [O, GN_GROUPS])
        gmaskT = ld(gmaskT_d, [GN_GROUPS, O])
        identb = ld(identb_d, [128, 128], BF)
        onesbc = ld(onesbc_d, [1, 128], BF)
        onescol = ld(onescol_d, [C, 1], BF)
        iscale = ld(iscale_d, [1, 1])
        zz = ld(zz_d, [2, MPP * MPP], BF)

        # tanh(identity_scale) broadcast to all partitions (once)
        tsc = sm.tile([1, 1], F32, tag="tsc")
        nc.scalar.activation(tsc[:], iscale[:], AF.Tanh)
        tcol = sm.tile([128, 1], F32, tag="tcol")
        nc.gpsimd.partition_broadcast(tcol[:], tsc[:], 128)

        def window_ap(tile2p, m, off, dims):
            """Manual strided AP at free-offset `off` of row m of a [2,N] tile."""
            base = tile2p[m:m + 1, off:]
            apl = [list(p) for p in base.ap]
            new = [apl[0]] + [[s, c] for (s, c) in dims]
            return dataclasses.replace(base, ap=new)

        mp = wrk.tile([2, MPP * MPP], BF, tag="mp", bufs=1)
        nc.gpsimd.dma_start(mp[:], zz[:])

        gstate = []
        for b in range(BL):
            # ---------------- Phase G: load + gates ----------------
            xf = big.tile([C, L], F32, tag="xf", bufs=2)
            for q in range(4):
                nc.sync.dma_start(xf[:, q * 1024:(q + 1) * 1024],
                                  x_d[b][:, q * 1024:(q + 1) * 1024])

            # per-channel sums of x (for p = mean): chunked ACT copies w/ accum
            pcols = sm.tile([C, NCH], F32, tag="pcols", bufs=2)
            for j in range(NCH):
                trash = wrk.tile([C, CHL], BF, tag="trash", bufs=1)
                nc.scalar.activation(trash[:], xf[:, j * CHL:(j + 1) * CHL], AF.Copy,
                                     accum_out=pcols[:, j:j + 1])
            psums = sm.tile([C, 1], F32, tag="psums", bufs=2)
            nc.vector.tensor_reduce(psums[:], pcols[:], axis=AX.X, op=ALU.add)

            # g = sigmoid(silu(p @ gc_w1.T) @ gc_w2.T)   (1/L folded into wgc1)
            p1 = paux.tile([HID, 1], F32, tag="aux")
            nc.tensor.matmul(p1[:], wgc1[:], psums[:], start=True, stop=True)
            s1 = sm.tile([HID, 1], F32, tag="s1", bufs=2)
            nc.scalar.activation(s1[:], p1[:], AF.Sigmoid)
            h1 = sm.tile([HID, 1], F32, tag="h1", bufs=2)
            nc.vector.tensor_mul(h1[:], s1[:], p1[:])
            p2 = paux.tile([C, 1], F32, tag="aux")
            nc.tensor.matmul(p2[:], wgc2[:], h1[:], start=True, stop=True)
            g = sm.tile([C, 1], F32, tag="g", bufs=2)
            nc.scalar.activation(g[:], p2[:], AF.Sigmoid)

            # channel gate: pc = g * p (1/L folded into wcg1)
            pc = sm.tile([C, 1], F32, tag="pc", bufs=2)
            nc.vector.tensor_mul(pc[:], g[:], psums[:])
            p3 = paux.tile([CGD, 1], F32, tag="aux")
            nc.tensor.matmul(p3[:], wcg1[:], pc[:], start=True, stop=True)
            s2 = sm.tile([CGD, 1], F32, tag="s2", bufs=2)
            nc.scalar.activation(s2[:], p3[:], AF.Sigmoid, bias=bcg1[:])
            h2 = sm.tile([CGD, 1], F32, tag="h2", bufs=2)
            nc.vector.scalar_tensor_tensor(out=h2[:], in0=p3[:], scalar=bcg1[:],
                                           in1=s2[:], op0=ALU.add, op1=ALU.mult)
            p4 = paux.tile([C, 1], F32, tag="aux")
            nc.tensor.matmul(p4[:], wcg2[:], h2[:], start=True, stop=True)
            cat = sm.tile([C, 1], F32, tag="cat", bufs=2)
            nc.scalar.activation(cat[:], p4[:], AF.Sigmoid, bias=bcg2[:])
            gc = sm.tile([C, 1], F32, tag="gcv", bufs=2)
            nc.vector.tensor_mul(gc[:], g[:], cat[:])
            if debug:
                gco = sm.tile([C, 2], F32, tag="gco")
                nc.vector.tensor_copy(gco[:, 0:1], g[:])
                nc.vector.tensor_copy(gco[:, 1:2], cat[:])
                nc.sync.dma_start(dbg["gc"][b], gco[:])

            # ---------------- spatial gate ----------------
            # xg = x * g in bf16; transposes vs identity; per-chunk reduces
            xg = big.tile([C, L], BF, tag="xg", bufs=1)
            nc.vector.tensor_scalar_mul(xg[:], xf[:], g[:])
            # mean channel: sum over C via ones-matmul -> msb [1, L] bf16
            msb = wrk.tile([1, L], BF, tag="msb", bufs=1)
            for j in range(NCH):
                pmn = pbc.tile([1, CHL], F32, tag="bc")
                nc.tensor.matmul(pmn[:], onescol[:], xg[:, j * CHL:(j + 1) * CHL],
                                 start=True, stop=True)
                nc.scalar.activation(msb[0:1, j * CHL:(j + 1) * CHL], pmn[:], AF.Copy)
            # max channel: 32 transposes + DVE max
            mx = sm.tile([128, 32], F32, tag="mx", bufs=2)
            for t in range(32):
                tp_pool, tp_tag = (paux, "aux") if (t % 2 == 0) else (pbc, "bc")
                ptr = tp_pool.tile([128, 128], BF, tag=tp_tag)
                nc.tensor.transpose(ptr[:], xg[:, t * 128:(t + 1) * 128], identb[:])
                nc.vector.tensor_reduce(mx[:, t:t + 1], ptr[:], axis=AX.X, op=ALU.max)
            mxb = sm.tile([128, 32], BF, tag="mxb", bufs=2)
            nc.vector.tensor_copy(mxb[:], mx[:])
            pmt = paux.tile([32, 128], BF, tag="aux")
            nc.tensor.transpose(pmt[:], mxb[:], identb[:])
            sbm = sm.tile([32, 128], BF, tag="sbm", bufs=2)
            nc.scalar.activation(sbm[:], pmt[:], AF.Copy)

            # padded maps [2, 70*70] bf16: row m=0 mean(sum), m=1 max
            qeng = [nc.sync, nc.gpsimd]
            qi = 0
            for hp in range(2):
                src = sbm[0:32, hp * 64:(hp + 1) * 64]
                base0 = (3 + hp) * MPP + 3
                seg = mp[1:2, base0:base0 + 32 * 2 * MPP]
                dst = seg.rearrange("p (t q) -> p t q", q=2 * MPP)[:, :, 0:64]
                qeng[qi % 2].dma_start(dst, src); qi += 1
            dmean = window_ap(mp, 0, 3 * MPP + 3, [(MPP, 64), (1, 64)])
            qeng[qi % 2].dma_start(dmean, msb[0:1, :]); qi += 1

            # patches [98, 4096] bf16 via 2-stage DMA (16 DMAs instead of 98):
            # stage 1: MB[m*7+a, :] = mp[m, a*70 : a*70+4480]  (row-shift replicas)
            MB = wrk.tile([14, 4480], BF, tag="MB", bufs=1)
            for m in range(2):
                src = window_ap(mp, m, 0, [(MPP, 7), (1, 4480)])
                qeng[m % 2].dma_start(MB[m * 7:(m + 1) * 7, :], src)
            # stage 2: per (m,b): pat[m*49+a*7+b, l] = MB[m*7+a, b + (h*70+w)]
            # pat row order: p = b*14 + m*7 + a  (one DMA per b, both maps)
            pat = wrk.tile([98, L], BF, tag="pat", bufs=1)
            qeng3 = [nc.sync, nc.gpsimd, nc.scalar]
            for bb in range(7):
                sb_ = MB[0:14, bb:]
                apl = [list(p) for p in sb_.ap]
                src = dataclasses.replace(sb_, ap=[apl[0], [MPP, 64], [1, 64]])
                dst = pat[bb * 14:(bb + 1) * 14, :]
                qeng3[bb % 3].dma_start(dst, src)

            # s-conv + sigmoid -> s_att [1, L] bf16
            satt = wrk.tile([1, L], BF, tag="satt", bufs=1)
            for j in range(NCH):
                pss = paux.tile([1, CHL], F32, tag="aux")
                nc.tensor.matmul(pss[:], wsg[:], pat[:, j * CHL:(j + 1) * CHL],
                                 start=True, stop=True)
                nc.scalar.activation(satt[0:1, j * CHL:(j + 1) * CHL], pss[:], AF.Sigmoid)
            if debug:
                sattf = wrk.tile([1, L], F32, tag="sattf")
                nc.vector.tensor_copy(sattf[:], satt[:])
                nc.sync.dma_start(dbg["satt"][b], sattf[:])

            # ---------------- Phase X2: gated input into padded buffer ----------------
            x2p = big.tile([C, HP * HP], BF, tag="x2p", bufs=2)
            x2v = x2p[:].rearrange("c (h q) -> c h q", q=HP)
            # zero only the pad strips (top/bottom rows, left/right cols)
            nc.vector.memset(x2v[:, 0:2, :], 0.0)
            nc.vector.memset(x2v[:, 66:68, :], 0.0)
            nc.vector.memset(x2v[:, 2:66, 0:2], 0.0)
            nc.vector.memset(x2v[:, 2:66, 66:68], 0.0)
            xfv = xf[:].rearrange("c (h q) -> c h q", q=W)
            x2cols = sm.tile([C, NCH], F32, tag="x2cols", bufs=2)
            for j in range(NCH):
                pb = pbc.tile([C, CHL], F32, tag="bc")
                nc.tensor.matmul(pb[:], onesbc[:], satt[0:1, j * CHL:(j + 1) * CHL],
                                 start=True, stop=True)
                pbv = pb[:].rearrange("c (h q) -> c h q", q=W)
                dst = x2v[:, 8 * j + 2: 8 * j + 10, 2:2 + W]
                nc.vector.scalar_tensor_tensor(
                    out=dst, in0=xfv[:, 8 * j:8 * j + 8, :], scalar=gc[:],
                    in1=pbv, op0=ALU.mult, op1=ALU.mult,
                    accum_out=x2cols[:, j:j + 1])
            x2sums = sm.tile([C, 1], F32, tag="x2sums", bufs=2)
            nc.vector.tensor_reduce(x2sums[:], x2cols[:], axis=AX.X, op=ALU.add)
            if debug:
                nc.sync.dma_start(dbg["x2s"][b], x2sums[:])
            # gf replicated 4x (wrg tiled host-side, 1/L folded)
            pgf = paux.tile([128, 1], F32, tag="aux")
            nc.tensor.matmul(pgf[:], wrg[:], x2sums[:], start=True, stop=True)
            sgf = sm.tile([128, 1], F32, tag="sgf", bufs=2)
            nc.scalar.activation(sgf[:], pgf[:], AF.Sigmoid)
            gf = sm.tile([128, 1], F32, tag="gf", bufs=2)
            nc.vector.tensor_mul(gf[:], sgf[:], pgf[:])
            if debug:
                nc.sync.dma_start(dbg["gf"][b], gf[:])

            gstate.append((xf, x2v, gf))

        for b in range(BL):
            xf, x2v, gf = gstate[b]
            # ---------------- Phase R: router conv + silu for all chunks ----------------
            lfimg = big.tile([128, L], BF, tag="lfimg", bufs=1)
            for j in range(NCH):
                pr = pbig.tile([128, CHL], F32, tag="mm")
                for ti, (di, dj) in enumerate((a, c2) for a in (0, 2, 4) for c2 in (0, 2, 4)):
                    rhs = x2v[:, 8 * j + di: 8 * j + di + 8, dj:dj + W]
                    nc.tensor.matmul(pr[:], wrc[:, ti * O:(ti + 1) * O], rhs,
                                     start=(ti == 0), stop=(ti == 8))
                lf = wrk.tile([128, CHL], BF, tag="lf", bufs=2)
                nc.scalar.activation(lf[:], pr[:], AF.Silu, bias=brc[:])
                nc.vector.tensor_scalar_add(lfimg[:, j * CHL:(j + 1) * CHL], lf[:], gf[:])

            # ---------------- Phase M: alpha + main + epilogue ----------------
            outpre = big.tile([O, L], F32, tag="outpre", bufs=2)
            sumf = sm.tile([O, NCH], F32, tag="sumf", bufs=2)
            sumsq = sm.tile([O, NCH], F32, tag="sumsq", bufs=2)
            for j in range(NCH):
                # alpha: 4 row-tiled matmuls K=32 -> exp
                es = []
                for k in range(K):
                    pa = pbig.tile([O, CHL], F32, tag="mm")
                    nc.tensor.matmul(pa[:], wgate[32 * k:32 * (k + 1), :],
                                     lfimg[32 * k:32 * (k + 1), j * CHL:(j + 1) * CHL],
                                     start=True, stop=True, tile_position=(32 * k, 0))
                    e = wrk.tile([O, CHL], BF, tag="e", bufs=5)
                    nc.scalar.activation(e[:], pa[:], AF.Exp,
                                         bias=bgate[:, k:k + 1], scale=1.0 / TAU)
                    es.append(e)

                # main: 4 experts x 9 taps; products read PSUM directly
                pms = []
                for k in range(K):
                    pm = pbig.tile([O, CHL], F32, tag="mm")
                    for ti, (di, dj) in enumerate((a, c2) for a in (1, 2, 3) for c2 in (1, 2, 3)):
                        rhs = x2v[:, 8 * j + di: 8 * j + di + 8, dj:dj + W]
                        nc.tensor.matmul(pm[:], wexp[:, (k * 9 + ti) * O:(k * 9 + ti + 1) * O],
                                         rhs, start=(ti == 0), stop=(ti == 8))
                    pms.append(pm)

                # epilogue: out = (sum_k e_k*res_k) / (sum_k e_k)
                t1 = wrk.tile([O, CHL], BF, tag="t1", bufs=2)
                nc.gpsimd.tensor_add(t1[:], es[0][:], es[1][:])
                t2 = wrk.tile([O, CHL], BF, tag="t2", bufs=2)
                nc.gpsimd.tensor_add(t2[:], es[2][:], es[3][:])
                Ef = wrk.tile([O, CHL], F32, tag="Ef", bufs=1)
                nc.gpsimd.tensor_add(Ef[:], t1[:], t2[:])
                rE = wrk.tile([O, CHL], F32, tag="rE", bufs=1)
                nc.vector.reciprocal_approx_fast(out=rE[:], in_=Ef[:])
                ps0 = wrk.tile([O, CHL], BF, tag="ps0", bufs=2)
                nc.vector.tensor_mul(ps0[:], es[0][:], pms[0][:])
                ps1 = wrk.tile([O, CHL], BF, tag="ps1", bufs=2)
                nc.vector.tensor_mul(ps1[:], es[1][:], pms[1][:])
                ps2 = wrk.tile([O, CHL], BF, tag="ps2", bufs=2)
                nc.vector.tensor_mul(ps2[:], es[2][:], pms[2][:])
                ps3 = wrk.tile([O, CHL], BF, tag="ps3", bufs=2)
                nc.vector.tensor_mul(ps3[:], es[3][:], pms[3][:])
                q1 = wrk.tile([O, CHL], BF, tag="q1", bufs=2)
                nc.gpsimd.tensor_add(q1[:], ps0[:], ps1[:])
                q2 = wrk.tile([O, CHL], BF, tag="q2", bufs=2)
                nc.gpsimd.tensor_add(q2[:], ps2[:], ps3[:])
                Pt = wrk.tile([O, CHL], F32, tag="Pt", bufs=1)
                nc.vector.tensor_add(Pt[:], q1[:], q2[:])
                oc = outpre[:, j * CHL:(j + 1) * CHL]
                nc.vector.scalar_tensor_tensor(
                    out=oc, in0=Pt[:], scalar=1.0, in1=rE[:],
                    op0=ALU.mult, op1=ALU.mult, accum_out=sumf[:, j:j + 1])
                tr2 = wrk.tile([O, CHL], BF, tag="tr2", bufs=2)
                nc.vector.scalar_tensor_tensor(
                    out=tr2[:], in0=oc, scalar=1.0, in1=oc,
                    op0=ALU.mult, op1=ALU.mult, accum_out=sumsq[:, j:j + 1])
            if debug:
                opf = wrk.tile([O, L], F32, tag="opf")
                nc.vector.tensor_copy(opf[:], outpre[:])
                nc.sync.dma_start(dbg["outpre"][b], opf[:])

            # ---------------- Tail: GroupNorm stats + SE + final ----------------
            SS = sm.tile([O, 2], F32, tag="SS", bufs=2)
            nc.vector.tensor_reduce(SS[:, 0:1], sumf[:], axis=AX.X, op=ALU.add)
            nc.vector.tensor_reduce(SS[:, 1:2], sumsq[:], axis=AX.X, op=ALU.add)
            psg = paux.tile([GN_GROUPS, 2], F32, tag="aux")
            nc.tensor.matmul(psg[:], gmask[:], SS[:], start=True, stop=True)
            # gmask pre-scaled by 1/(16*L) host-side: psg = [mu_g, E[x^2]_g]
            mr = sm.tile([GN_GROUPS, 2], F32, tag="mr", bufs=2)
            nc.scalar.copy(mr[:], psg[:])
            musq = sm.tile([GN_GROUPS, 1], F32, tag="musq", bufs=2)
            nc.vector.tensor_mul(musq[:], mr[:, 0:1], mr[:, 0:1])
            varg = sm.tile([GN_GROUPS, 1], F32, tag="varg", bufs=2)
            nc.vector.scalar_tensor_tensor(out=varg[:], in0=musq[:], scalar=-1.0,
                                           in1=mr[:, 1:2], op0=ALU.mult, op1=ALU.add)
            nc.vector.tensor_scalar_add(varg[:], varg[:], EPS)
            sdg = sm.tile([GN_GROUPS, 1], F32, tag="sdg", bufs=2)
            nc.scalar.activation(sdg[:], varg[:], AF.Sqrt)
            nc.vector.reciprocal(mr[:, 1:2], sdg[:])
            psc = paux.tile([O, 2], F32, tag="aux")
            nc.tensor.matmul(psc[:], gmaskT[:], mr[:], start=True, stop=True)
            mrc = sm.tile([O, 2], F32, tag="mrc", bufs=2)
            nc.scalar.copy(mrc[:], psc[:])
            muc, rstdc = mrc[:, 0:1], mrc[:, 1:2]

            # SE input: ps = (mean_c - mu)*rstd*gamma + beta
            m1 = sm.tile([O, 1], F32, tag="m1", bufs=2)
            nc.vector.tensor_scalar_mul(m1[:], SS[:, 0:1], 1.0 / L)
            d1 = sm.tile([O, 1], F32, tag="d1", bufs=2)
            nc.vector.tensor_sub(d1[:], m1[:], muc)
            d2 = sm.tile([O, 1], F32, tag="d2", bufs=2)
            nc.vector.tensor_mul(d2[:], d1[:], rstdc)
            psin = sm.tile([O, 1], F32, tag="psin", bufs=2)
            nc.vector.scalar_tensor_tensor(out=psin[:], in0=d2[:], scalar=gng[:],
                                           in1=gnb[:], op0=ALU.mult, op1=ALU.add)
            pse1 = paux.tile([SE, 1], F32, tag="aux")
            nc.tensor.matmul(pse1[:], wse1[:], psin[:], start=True, stop=True)
            sse = sm.tile([SE, 1], F32, tag="sse", bufs=2)
            nc.scalar.activation(sse[:], pse1[:], AF.Sigmoid, bias=bse1[:])
            hse = sm.tile([SE, 1], F32, tag="hse", bufs=2)
            nc.vector.scalar_tensor_tensor(out=hse[:], in0=pse1[:], scalar=bse1[:],
                                           in1=sse[:], op0=ALU.add, op1=ALU.mult)
            pse2 = paux.tile([O, 1], F32, tag="aux")
            nc.tensor.matmul(pse2[:], wse2[:], hse[:], start=True, stop=True)
            sev = sm.tile([O, 1], F32, tag="sev", bufs=2)
            nc.scalar.activation(sev[:], pse2[:], AF.Sigmoid, bias=bse2[:])

            # A = rstd*gamma*se ; Bv = (beta - mu*rstd*gamma)*se + bias
            rg_ = sm.tile([O, 1], F32, tag="rg_", bufs=2)
            nc.vector.tensor_mul(rg_[:], rstdc, gng[:])
            Acol = sm.tile([O, 1], F32, tag="Acol", bufs=2)
            nc.vector.tensor_mul(Acol[:], rg_[:], sev[:])
            b1_ = sm.tile([O, 1], F32, tag="b1_", bufs=2)
            nc.vector.tensor_mul(b1_[:], muc, rg_[:])
            b2_ = sm.tile([O, 1], F32, tag="b2_", bufs=2)
            nc.vector.tensor_sub(b2_[:], gnb[:], b1_[:])
            Bcol = sm.tile([O, 1], F32, tag="Bcol", bufs=2)
            nc.vector.scalar_tensor_tensor(out=Bcol[:], in0=b2_[:], scalar=sev[:],
                                           in1=biaso[:], op0=ALU.mult, op1=ALU.add)

            # final: out = A*outpre + Bv + tanh(iscale)*x
            for j in range(NCH):
                tmf = wrk.tile([O, CHL], F32, tag="tmf", bufs=1)
                nc.scalar.activation(tmf[:], outpre[:, j * CHL:(j + 1) * CHL],
                                     AF.Identity, bias=Bcol[:], scale=Acol[:])
                ofin = wrk.tile([O, CHL], F32, tag="ofin", bufs=2)
                nc.vector.scalar_tensor_tensor(
                    out=ofin[:], in0=xf[:, j * CHL:(j + 1) * CHL], scalar=tcol[:],
                    in1=tmf[:], op0=ALU.mult, op1=ALU.add)
                nc.sync.dma_start(out_d[b][:, j * CHL:(j + 1) * CHL], ofin[:])

    nc.finalize()
    return nc


def _prep_consts(inputs):
    i = inputs
    s_bn = (i["bn_gamma"] / np.sqrt(i["bn_var"] + EPS)).astype(np.float32)
    b_rc1 = (i["bn_beta"] - i["bn_mean"] * s_bn).astype(np.float32)
    rcw = (i["rc_w"] * s_bn[:, None, None, None]).astype(np.float32)   # [32,128,3,3]
    # router lhsT per tap, tiled 4x along M: [C, 9, 128]
    wrc = np.tile(rcw.transpose(2, 3, 1, 0), (1, 1, 1, 4))             # [3,3,128,128]
    wrc = np.ascontiguousarray(wrc.reshape(9, C, 4 * HID).transpose(1, 0, 2).reshape(C, 9 * O))

    delta = (i["delta_weight"] * i["kernel_scale"]).astype(np.float32)
    wex = i["base_weight"][None] + delta                               # [4,O,C,3,3]
    wex = wex.transpose(2, 0, 3, 4, 1)                                 # [C,4,3,3,O]
    wex = np.ascontiguousarray(wex.reshape(C, K * 9 * O))

    wgate = np.concatenate([i["gate_w"][k * O:(k + 1) * O, :].T for k in range(K)], 0)
    bgate = np.ascontiguousarray((i["gate_b"].reshape(K, O).T / TAU))

    wsg = np.empty((98, 1), np.float32)
    w2 = i["sg_w"][0].transpose(2, 0, 1)           # [b, m, a]
    wsg[:, 0] = w2.reshape(98).astype(np.float32)
    wsg[0::14, 0] /= C; wsg[1::14, 0] /= C; wsg[2::14, 0] /= C
    wsg[3::14, 0] /= C; wsg[4::14, 0] /= C; wsg[5::14, 0] /= C; wsg[6::14, 0] /= C

    gmask = np.zeros((O, GN_GROUPS), np.float32)
    gmask[np.arange(O), np.arange(O) // (O // GN_GROUPS)] = 1.0

    c = {
        "wexp": wex.astype(bf16),
        "wrc": wrc.astype(bf16),
        "brc": np.tile(b_rc1, 4).reshape(O, 1).astype(np.float32),
        "wgate": wgate.astype(bf16),
        "bgate": bgate.astype(np.float32),
        "wgc1": np.ascontiguousarray(i["gc_w1"].T / L).astype(np.float32),
        "wgc2": np.ascontiguousarray(i["gc_w2"].T).astype(np.float32),
        "wcg1": np.ascontiguousarray(i["cg_w1"].T / L).astype(np.float32),
        "bcg1": i["cg_b1"].reshape(CGD, 1).astype(np.float32),
        "wcg2": np.ascontiguousarray(i["cg_w2"].T).astype(np.float32),
        "bcg2": i["cg_b2"].reshape(C, 1).astype(np.float32),
        "wrg": np.tile(np.ascontiguousarray(i["rg_w"].T) / L, (1, 4)).astype(np.float32),
        "wsg": wsg.astype(bf16),
        "wse1": np.ascontiguousarray(i["se_w1"].T).astype(np.float32),
        "bse1": i["se_b1"].reshape(SE, 1).astype(np.float32),
        "wse2": np.ascontiguousarray(i["se_w2"].T).astype(np.float32),
        "bse2": i["se_b2"].reshape(O, 1).astype(np.float32),
        "gng": i["gn_gamma"].reshape(O, 1).astype(np.float32),
        "gnb": i["gn_beta"].reshape(O, 1).astype(np.float32),
        "biaso": i["bias"].reshape(O, 1).astype(np.float32),
        "gmask": gmask / float(16 * L),
        "gmaskT": np.ascontiguousarray(gmask.T),
        "identb": np.eye(128, dtype=bf16),
        "onesbc": np.ones((1, 128), bf16),
        "onescol": np.ones((C, 1), bf16),
        "iscale": np.asarray(i["identity_scale"], np.float32).reshape(1, 1),
        "zz": np.zeros((2, MPP * MPP), bf16),
    }
    return c


def kernel(**inputs):
    inputs = {k: np.asarray(v) for k, v in inputs.items()}
    consts = _prep_consts(inputs)
    x = inputs["x"].astype(np.float32).reshape(B, C, L)
    in_maps = []
    for i in range(NCORES):
        m = dict(consts)
        m["x"] = np.ascontiguousarray(x[i * BL:(i + 1) * BL])
        in_maps.append(m)

    if "nc" not in _CACHE:
        _CACHE["nc"] = _build()
    from concourse.bass_utils import run_bass_kernel_spmd
    res = run_bass_kernel_spmd(_CACHE["nc"], in_maps, core_ids=list(range(NCORES)))
    out = np.concatenate([res.results[i]["out"] for i in range(NCORES)], axis=0)
    return np.ascontiguousarray(out.reshape(B, O, H, W).astype(np.float32))


if __name__ == "__main__":
    rng = np.random.default_rng(0)
    pass
